# revision 35
# baseline (speedup 1.0000x reference)
"""Trainium2 Bass kernel for nn_CholeskyConstraintLayer.

Maps x:(B,16) f32 -> rho:(B,4,4,2) f32 where rho = L L^dagger / (trace + eps),
L lower-triangular complex 4x4 built from x (softplus diagonal, raw re/im
off-diagonals).

PLANAR (SoA) design: the host stages each (core, tile) block of samples as a
plane-major (P, 16, F) array -- a pure layout transpose -- so that on-chip
every operand is a stride-1 run of F samples.  That keeps every DVE
tensor_tensor in the 2x fp16 fast mode (the cost model requires the last AP
dim to be packed for ALL operands) and lets per-sample broadcasts (rcp, dr)
ride outer stride-0 AP dims, which do not break the fast mode.

x plane order (natural tri layout): [d0, r10,i10, d1, r20,i20, r21,i21, d2,
r30,i30, r31,i31, r32,i32, d3]; d* get softplus.  The stride-2 (r,i)
interleave gives regular AP patterns: R2=(r20,r21)@{4,6}, I2=(i20,i21)@{5,7},
R3=(r30,r31,r32)@{9,11,13}, I3=(i30,i31,i32)@{10,12,14}, R1=(r10,d1)@{1,3}.

Per-sample math (22 products, 31 adds, softplus, 16 squares, recip, 16 norm
muls, 6 negations):
  re21 = r20*r10 + r21*d1 + i20*i10       im21 = i20*r10 + i21*d1 - r20*i10
  re31/im31 analogous with row 3;         re32 = R3.(r20,r21,d2) + (i30,i31).I2
  im32 = I3.(r20,r21,d2) - (r30,r31).I2
  qii = row sums of squares; trace = q00+q11+q22+q33 (+eps); rho *= 1/trace
  col0: (re_i0, im_i0) = (r_i0, i_i0) * d0 / trace  via dr = d0*rcp

Engine split: DVE does products + add-trees + trace/reciprocal (fp32) +
rcp cast + dr/q00/col0; ACT does softplus (Exp,Ln), the 16 squares and the
col0-imag negations; Pool does the q/dots normalises (tensor_tensor with the
rcp broadcast riding an outer stride-0 dim), the dot-imag negations, and the
L2a/L2b dot finishers (their outputs feed only Pool, so deferring them to S3
takes them off the DVE critical path without an in-order-queue stall).
Emission is a 4-stage software pipeline, one tile per stage per round:
S0 dma-in | S1 ACT softplus+squares | S2 DVE compute | S3 Pool norm + negs +
dma-out (3 chunks for big tiles so draining starts early).  The last
TAIL_DVE tiles run their norms/negs on DVE so the kernel end is not
serialised behind Pool's queue.

Output is 22 fp16 planes per tile: [q00, re10,im10,re20,im20,re30,im30,
q11,q22,q33, re21,re31,re32, im21,im31,im32, nim10,nim20,nim30, nim21,
nim31,nim32].  The host only gathers/zero-fills these into the (B,4,4,2)
f32 layout; all arithmetic happens on device.
"""

import numpy as np

P = 128
EPS = 1e-8
N_CORES = 8
BATCH = 1_000_000
SPP = 977  # samples per partition; P*SPP*N_CORES = 1000448 >= BATCH
F_LIST = [150, 300, 280, 215, 32]  # sum = SPP; descending taper hides tails
TAIL_DVE = 2  # trailing tiles whose normalise/negs run on DVE instead of Pool
NEG_MODE = "post"   # 'post': nim = im_norm * -1 ; 'nrcp': nim = im_unnorm * -rcp
NEGC_ENGINE = "act"  # engine for col0-imag negation on non-tail tiles
CHUNK_A_MIN = 128    # min F for a separate early chunk-A out DMA
CHUNK_C = True       # separate chunk-C DMA for negated planes
COL0_POOL = 0        # how many col0 planes (0/4/6/7=incl q00) run on Pool
DMAQ_POOL = False    # issue mid-tile out-DMAs from Pool (SWDGE) vs SP (HWDGE)
SQ_BUFS = 3
OUT_BUFS = 3
SPLIT_IN0 = True     # split tile0's in-DMA into two plane chunks
SQ0_DVE = False      # tile0's squares on DVE instead of ACT
RCP32_DIRECT = False  # Pool norms read fp32 rcp directly (no fp16 cast)
M4_POOL = False      # M4a/M4b (32-dot finishers, Pool-only consumers) on Pool
L2_POOL = True       # L2a/L2b (21/31-dot finishers) on Pool
NEGD_DVE = False     # dot-imag negation on DVE (4x) instead of Pool
S_CORE = P * SPP
S_PAD = S_CORE * N_CORES

IN_W = 16   # fp16 planes per sample in
OUT_W = 22  # fp16 planes per sample out

# out22 plane order: [q00, re10,im10, re20,im20, re30,im30, q11,q22,q33,
#  re21,re31,re32, im21,im31,im32, nim10,nim20,nim30, nim21,nim31,nim32]
# out plane -> rho flat-32 expansion (host): rho32[k] = out22[EXP_SRC[k]],
# EXP_SRC=-1 -> 0.
EXP_SRC = np.full(32, -1, dtype=np.int64)
for flat, src in {
    0: 0, 10: 7, 20: 8, 30: 9,
    8: 1, 9: 2, 2: 1, 3: 16,
    16: 3, 17: 4, 4: 3, 5: 17,
    24: 5, 25: 6, 6: 5, 7: 18,
    18: 10, 19: 13, 12: 10, 13: 19,
    26: 11, 27: 14, 14: 11, 15: 20,
    28: 12, 29: 15, 22: 12, 23: 21,
}.items():
    EXP_SRC[flat] = src

_NC_CACHE = {}


def _emit(tc, x_ap, out_ap, f_list):
    import concourse.bass as bass
    import concourse.mybir as mybir
    from contextlib import ExitStack

    nc = tc.nc
    f16 = mybir.dt.float16
    f32 = mybir.dt.float32
    A = mybir.AluOpType
    ACT = mybir.ActivationFunctionType

    def pap(tile, F, p0, dims=()):
        """Plane-pattern AP on a (P, nplanes*F) tile: outer dims in plane
        units [stride, count], innermost packed [1, F]."""
        v = tile[:, :]
        return bass.AP(tensor=v.tensor, offset=v.offset + p0 * F,
                       ap=[list(v.ap[0])] + [[s * F, c] for s, c in dims]
                       + [[1, F]])

    with ExitStack() as ctx:
        tp = lambda name, bufs: ctx.enter_context(
            tc.tile_pool(name=name, bufs=bufs))
        wpool = tp("w", 4 if COL0_POOL else 3)  # S0..S2 (+Pool lag)
        sqpool = tp("sq", SQ_BUFS)  # alive S1..S2
        tppool = tp("tp", 3 if L2_POOL else 2)
        scpool = tp("sc", 3 if (M4_POOL or L2_POOL) else 2)
        qcpool = tp("qc", 3)   # S2..S3 (Q planes read by Pool N1)
        dqpool = tp("dq", 3)   # S2..S3: dots, rcp16, dr
        c32pool = tp("c32", 3 if RCP32_DIRECT else 2)
        opool = tp("out", OUT_BUFS)  # written S2 (col0) + S3

        offs = []
        o = 0
        for F in f_list:
            offs.append(o)
            o += F
        states = [dict(ti=i, F=f_list[i], off=offs[i]) for i in range(len(f_list))]

        def s0_dma_in(st):
            F, off = st["F"], st["off"]
            w_t = wpool.tile([P, 16 * F], f16, tag="w", name=f"w{st['ti']}")
            W = lambda p0, dims=(): pap(w_t, F, p0, dims)
            if st["ti"] == 0 and SPLIT_IN0:
                # split tile0's load so softplus(d0,d1) starts before the
                # full tile arrives (shorter pipeline fill)
                for p0, np_ in ((0, 8), (8, 8)):
                    xin = bass.AP(tensor=x_ap.tensor,
                                  offset=IN_W * off + p0 * F,
                                  ap=[[IN_W * SPP, P], [1, np_ * F]])
                    nc.sync.dma_start(W(p0, [[1, np_]]), xin)
            else:
                xin = bass.AP(tensor=x_ap.tensor, offset=IN_W * off,
                              ap=[[IN_W * SPP, P], [1, IN_W * F]])
                nc.sync.dma_start(w_t[:, :], xin)
            st["W"] = W

        def s1_act(st):
            F, W = st["F"], st["W"]
            # softplus in place on d-planes {0,3,8,15}
            # (exp scratch = sq planes 0..3, overwritten later by Square)
            sq_t = sqpool.tile([P, 16 * F], f16, tag="sq", name=f"sq{st['ti']}")
            SQ = lambda p0, dims=(): pap(sq_t, F, p0, dims)
            nc.scalar.activation(SQ(0, [[1, 2]]), W(0, [[3, 2]]), ACT.Exp)
            nc.scalar.activation(SQ(2, [[1, 2]]), W(8, [[7, 2]]), ACT.Exp)
            nc.scalar.activation(W(0, [[3, 2]]), SQ(0, [[1, 2]]), ACT.Ln,
                                 bias=1.0)
            nc.scalar.activation(W(8, [[7, 2]]), SQ(2, [[1, 2]]), ACT.Ln,
                                 bias=1.0)
            # squares of all 16 planes
            if st["ti"] == 0 and SQ0_DVE:
                nc.vector.tensor_tensor(SQ(0, [[1, 16]]), W(0, [[1, 16]]),
                                        W(0, [[1, 16]]), op=A.mult)
            else:
                nc.scalar.activation(SQ(0, [[1, 16]]), W(0, [[1, 16]]), ACT.Square)
            st["SQ"] = SQ

        def s2_dve(st):
            ti, F, W, SQ, off = st["ti"], st["F"], st["W"], st["SQ"], st["off"]
            tt = lambda dst, a, b, op: nc.vector.tensor_tensor(dst, a, b, op=op)
            # ---- products (22 els/sample), term planes TP[0:23]
            # TP: 0-7 Pa (re21 t0t1, im21 t0t1, re31 t0t1, im31 t0t1),
            #     8-13 Pb (re32 t0t1t2, im32 t0t1t2),
            #     14-18 Pd (im32s0@14, im32s1@15, [16 unused], re32e0@17, re32e1@18)
            #     19-22 Pc (im21s, re21t2, im31s, re31t2)
            tp_t = tppool.tile([P, 23 * F], f16, tag="tp", name=f"tp{ti}")
            TP = lambda p0, dims=(): pap(tp_t, F, p0, dims)
            # Pd, Pc first: no softplus dependency
            tt(TP(14, [[3, 2], [1, 2]]), W(9, [[1, 2], [2, 2]]), W(5, [[0, 2], [2, 2]]), A.mult)
            tt(TP(19, [[1, 4]]), W(4, [[5, 2], [1, 2]]), W(2, [[0, 2], [0, 2]]), A.mult)
            tt(TP(0, [[1, 4]]), W(4, [[1, 2], [2, 2]]), W(1, [[0, 2], [2, 2]]), A.mult)
            tt(TP(4, [[1, 4]]), W(9, [[1, 2], [2, 2]]), W(1, [[0, 2], [2, 2]]), A.mult)
            tt(TP(8, [[1, 6]]), W(9, [[1, 2], [2, 3]]), W(4, [[0, 2], [2, 3]]), A.mult)

            # ---- off-diag add tree -> dots dq[0:6]
            sc_np = 14 if L2_POOL else 10
            sc_t = scpool.tile([P, sc_np * F], f16, tag="sc", name=f"sc{ti}")
            SC = lambda p0, dims=(): pap(sc_t, F, p0, dims)
            dq_t = dqpool.tile([P, 9 * F], f16, tag="dq", name=f"dq{ti}")
            # dq planes: 0-5 dots (re21,re31,re32,im21,im31,im32), 6 rcp16,
            # 7 dr, 8 nrcp (= -rcp16, for the negated-imag outputs)
            DQ = lambda p0, dims=(): pap(dq_t, F, p0, dims)
            tail_t = ti >= len(f_list) - TAIL_DVE
            # L1: S[0:4] = (re21',im21',re31',im31')
            tt(SC(0, [[1, 4]]), TP(0, [[2, 4]]), TP(1, [[2, 4]]), A.add)
            if not (L2_POOL and not tail_t):
                tt(DQ(0, [[1, 2]]), SC(0, [[2, 2]]), TP(20, [[2, 2]]), A.add)
                tt(DQ(3, [[1, 2]]), SC(1, [[2, 2]]), TP(19, [[2, 2]]), A.subtract)
            # M13: (U0,U1,Vs,Ve) = TP{8,11,14,17} + TP{9,12,15,18}
            tt(SC(4, [[1, 4]]), TP(8, [[3, 4]]), TP(9, [[3, 4]]), A.add)
            # M2: U2 = U + (t2 of re32, im32)
            tt(SC(8, [[1, 2]]), SC(4, [[1, 2]]), TP(10, [[3, 2]]), A.add)
            if not (M4_POOL and not tail_t):
                tt(DQ(2), SC(8), SC(7), A.add)        # re32 = re32a + Ve
                tt(DQ(5), SC(9), SC(6), A.subtract)   # im32 = im32a - Vs
            st["SC"], st["TPf"] = SC, TP

            # ---- diag add tree (reuses sc planes 0..3 for B)
            # qc: 0 q11', 1 q22', 2 q33', 3 E, 4 q33p, 5 t1, 6 t2,
            #     7 q11, 8 q22, 9 q33
            qc_t = qcpool.tile([P, 10 * F], f16, tag="qc", name=f"qc{ti}")
            QC = lambda p0, dims=(): pap(qc_t, F, p0, dims)
            b0 = 10 if L2_POOL else 0  # avoid clobbering S before Pool's L2
            tt(SC(b0, [[1, 4]]), SQ(4, [[5, 2], [1, 2]]), SQ(6, [[5, 2], [1, 2]]), A.add)
            tt(QC(0, [[3, 2]]), SQ(1, [[12, 2]]), SQ(2, [[12, 2]]), A.add)
            tt(QC(1, [[1, 2]]), SC(b0, [[2, 2]]), SC(b0 + 1, [[2, 2]]), A.add)
            tt(QC(7, [[1, 2]]), QC(0, [[1, 2]]), SQ(3, [[5, 2]]), A.add)
            # fused: (q33p, t1) = (q33', q11) + (E, q22)
            tt(QC(4, [[1, 2]]), QC(2, [[5, 2]]), QC(3, [[5, 2]]), A.add)
            tt(QC(9), QC(4), SQ(15), A.add)
            tt(QC(6), QC(9), SQ(0), A.add)

            # ---- trace -> rcp (fp32), cast fp16 on ACT, dr + col0 on DVE
            c32_t = c32pool.tile([P, 2 * F], f32, tag="c32", name=f"c{ti}")
            trE = pap(c32_t, F, 0)
            rcp32 = pap(c32_t, F, 1)
            nc.vector.scalar_tensor_tensor(trE, QC(5), float(EPS), QC(6),
                                           op0=A.add, op1=A.add)
            nc.vector.reciprocal_approx_fast(rcp32, trE)
            if RCP32_DIRECT:
                # Pool has no fast mode, so its norms read rcp32 directly;
                # dr drops to 1x (mixed dtype) but the cast disappears.
                RCP = lambda dims=(): pap(c32_t, F, 1, dims)
                tt(DQ(7), W(0), RCP(), A.mult)  # dr = d0 * rcp (1x)
            else:
                RCP = lambda dims=(): pap(dq_t, F, 6, dims)
                nc.vector.tensor_copy(DQ(6), rcp32)
                tt(DQ(7), W(0), DQ(6), A.mult)  # dr = d0 * rcp
            if NEG_MODE == "nrcp":
                nc.vector.tensor_scalar_mul(DQ(8), RCP(), -1.0)  # nrcp
            st["RCP"] = RCP
            out_t = opool.tile([P, OUT_W * F], f16, tag="out", name=f"o{ti}")
            OUT = lambda p0, dims=(): pap(out_t, F, p0, dims)
            tail = ti >= len(f_list) - TAIL_DVE
            c0p = 0 if tail else COL0_POOL
            # (q00, re10, im10) = (d0, r10, i10) * dr ; then rows 2,3 pairs
            # (planes moved to Pool run in s3 to avoid queue head-of-line)
            if c0p < 7:
                tt(OUT(0, [[1, 3]]), W(0, [[1, 3]]), DQ(7, [[0, 3]]), A.mult)
            if c0p < 4:
                tt(OUT(3, [[1, 4]]), W(4, [[5, 2], [1, 2]]),
                   DQ(7, [[0, 2], [0, 2]]), A.mult)
            st["c0p"] = c0p
            if F >= CHUNK_A_MIN and not c0p:
                # DMA out chunk A: DVE-written planes 0..6
                odst = bass.AP(tensor=out_ap.tensor, offset=OUT_W * off,
                               ap=[[OUT_W * SPP, P], [1, 7 * F]])
                nc.sync.dma_start(odst, OUT(0, [[1, 7]]))
                st["chunkA"] = True
            else:
                st["chunkA"] = False
            st["DQ"], st["QC"], st["OUT"] = DQ, QC, OUT
            st["Wf"] = W

        def s3_tail(st):
            F, off = st["F"], st["off"]
            DQ, QC, OUT = st["DQ"], st["QC"], st["OUT"]
            tail = st["ti"] >= len(f_list) - TAIL_DVE
            if tail:
                # last tiles: run norms on DVE so the kernel end is not
                # serialised behind Pool's queue
                ntt = lambda dst, a, b: nc.vector.tensor_tensor(dst, a, b, op=A.mult)
                nts = nc.vector.tensor_scalar_mul
            else:
                ntt = lambda dst, a, b: nc.gpsimd.tensor_tensor(dst, a, b, op=A.mult)
                nts = nc.gpsimd.tensor_scalar_mul
            # col0 planes assigned to Pool (emitted here, not in s2)
            c0p, W, F_ = st["c0p"], st["Wf"], st["F"]
            gtt = lambda dst, a, b: nc.gpsimd.tensor_tensor(dst, a, b, op=A.mult)
            SC, TP = st["SC"], st["TPf"]
            gta = lambda dst, a, b, op: nc.gpsimd.tensor_tensor(dst, a, b, op=op)
            if L2_POOL and not tail:
                gta(DQ(0, [[1, 2]]), SC(0, [[2, 2]]), TP(20, [[2, 2]]), A.add)
                gta(DQ(3, [[1, 2]]), SC(1, [[2, 2]]), TP(19, [[2, 2]]), A.subtract)
            if M4_POOL and not tail:
                gta(DQ(2), SC(8), SC(7), A.add)
                gta(DQ(5), SC(9), SC(6), A.subtract)
            if c0p >= 7:
                gtt(OUT(0, [[1, 3]]), W(0, [[1, 3]]), DQ(7, [[0, 3]]))
            if c0p >= 4:
                gtt(OUT(3, [[1, 4]]), W(4, [[5, 2], [1, 2]]),
                    DQ(7, [[0, 2], [0, 2]]))
            # q-norm + dots-norm (rcp bcast in outer stride-0 dim)
            RCP = st["RCP"]
            ntt(OUT(7, [[1, 3]]), QC(7, [[1, 3]]), RCP([[0, 3]]))
            ntt(OUT(10, [[1, 6]]), DQ(0, [[1, 6]]), RCP([[0, 6]]))
            # chunk B: normalised planes 7..15 (0..15 if no chunk A)
            p0 = 7 if st["chunkA"] else 0
            chunk_c = (CHUNK_C == 2) or (CHUNK_C and st["ti"] < len(f_list) - 1)
            pend = OUT_W if not chunk_c else 16
            dmaq = nc.scalar if tail else (nc.gpsimd if DMAQ_POOL else nc.sync)
            odstB = bass.AP(tensor=out_ap.tensor, offset=OUT_W * off + p0 * F,
                            ap=[[OUT_W * SPP, P], [1, (pend - p0) * F]])
            dmaB = lambda: dmaq.dma_start(odstB, OUT(p0, [[1, pend - p0]]))
            if chunk_c:
                dmaB()
            # negations
            if NEG_MODE == "nrcp":
                ntt(OUT(19, [[1, 3]]), DQ(3, [[1, 3]]), DQ(8, [[0, 3]]))
            elif NEGD_DVE:
                nc.vector.tensor_scalar_mul(OUT(19, [[1, 3]]), OUT(13, [[1, 3]]), -1.0)
            else:
                nts(OUT(19, [[1, 3]]), OUT(13, [[1, 3]]), -1.0)
            if tail:
                nc.vector.tensor_scalar_mul(OUT(16, [[1, 3]]), OUT(2, [[2, 3]]), -1.0)
            elif NEGC_ENGINE == "act":
                nc.scalar.mul(OUT(16, [[1, 3]]), OUT(2, [[2, 3]]), -1.0)
            else:
                nc.gpsimd.tensor_scalar_mul(OUT(16, [[1, 3]]), OUT(2, [[2, 3]]), -1.0)
            if chunk_c:
                odstC = bass.AP(tensor=out_ap.tensor, offset=OUT_W * off + 16 * F,
                                ap=[[OUT_W * SPP, P], [1, 6 * F]])
                dmaq.dma_start(odstC, OUT(16, [[1, 6]]))
            else:
                dmaB()

        nt = len(f_list)
        for r in range(nt + 3):
            if r < nt:
                s0_dma_in(states[r])
            if 1 <= r < nt + 1:
                s1_act(states[r - 1])
            if 2 <= r < nt + 2:
                s2_dve(states[r - 2])
            if 3 <= r:
                s3_tail(states[r - 3])


def _patch_act_tables():
    """Force every ACT function onto one table set so the table-load pass
    emits a single load (Exp/Ln/Square/Copy are all natively co-resident in
    natural_log_exp_and_others -- verified by the harness rel-err check)."""
    import concourse.bacc as bacc
    from concourse.hw_specs import get_activation_tables as _orig

    if getattr(bacc, "_act_tables_patched", False):
        return

    def _patched(arch):
        t = _orig(arch)
        return {k: (v if k == "natural_log_exp_and_others" else set())
                for k, v in t.items()}

    bacc.get_activation_tables = _patched
    bacc._act_tables_patched = True


def _build_nc(f_list):
    import concourse.bacc as bacc
    import concourse.mybir as mybir
    import concourse.tile as tile

    _patch_act_tables()

    key = (tuple(f_list), TAIL_DVE, NEG_MODE, NEGC_ENGINE, CHUNK_A_MIN, CHUNK_C,
           COL0_POOL, DMAQ_POOL, SQ_BUFS, OUT_BUFS, SPLIT_IN0, SQ0_DVE,
           RCP32_DIRECT, M4_POOL, L2_POOL, NEGD_DVE)
    if key in _NC_CACHE:
        return _NC_CACHE[key]
    nc = bacc.Bacc("TRN2", target_bir_lowering=False, debug=False)
    x = nc.dram_tensor("x", (P, IN_W * SPP), mybir.dt.float16,
                       kind="ExternalInput")
    out = nc.dram_tensor("out", (P, OUT_W * SPP), mybir.dt.float16,
                         kind="ExternalOutput")
    with tile.TileContext(nc) as tc:
        with nc.allow_low_precision(reason="fp16 pipeline, rel-err budget 2e-2"):
            _emit(tc, x.ap(), out.ap(), f_list)
    nc.compile()
    _NC_CACHE[key] = nc
    return nc


def _stage_in(x):
    """(B,16) f32 -> per-core (P, 16*SPP) fp16, per-tile plane-major blocks.
    Pure layout (pad, reshape, transpose) + fp16 cast."""
    B = x.shape[0]
    xp = np.zeros((S_PAD, IN_W), dtype=np.float16)
    xp[:B] = x
    xr = xp.reshape(N_CORES, P, SPP, IN_W)
    parts = []
    off = 0
    for F in F_LIST:
        blk = xr[:, :, off:off + F, :].transpose(0, 1, 3, 2)
        parts.append(np.ascontiguousarray(blk).reshape(N_CORES, P, IN_W * F))
        off += F
    return np.concatenate(parts, axis=2)


def _unstage_out(res_list, B):
    """Per-core (P, 22*SPP) fp16 tile blocks -> (B, 4, 4, 2) f32 via the
    EXP_SRC gather (host does layout + zero-fill only)."""
    out = np.stack([r.reshape(P, OUT_W * SPP) for r in res_list], axis=0)
    parts = []
    off = 0
    for F in F_LIST:
        blk = out[:, :, OUT_W * off:OUT_W * (off + F)]
        blk = blk.reshape(N_CORES, P, OUT_W, F).transpose(0, 1, 3, 2)
        parts.append(blk)
        off += F
    o22 = np.concatenate(parts, axis=2).reshape(S_PAD, OUT_W)[:B]
    out32 = np.zeros((B, 32), dtype=np.float32)
    used = EXP_SRC >= 0
    out32[:, used] = o22[:, EXP_SRC[used]].astype(np.float32)
    return out32.reshape(B, 4, 4, 2)


def kernel(x, _trace=False):
    from concourse.bass_utils import run_bass_kernel_spmd

    x = np.ascontiguousarray(np.asarray(x, dtype=np.float32))
    B = x.shape[0]
    assert x.shape == (B, 16) and B <= S_PAD
    xs = _stage_in(x)
    nc = _build_nc(F_LIST)
    in_maps = [{"x": np.ascontiguousarray(xs[i])} for i in range(N_CORES)]
    res = run_bass_kernel_spmd(nc, in_maps, core_ids=list(range(N_CORES)),
                               trace=_trace)
    result = _unstage_out([r["out"] for r in res.results], B)
    if _trace:
        return result, res
    return result


# revision 36
# speedup vs baseline: 1.0076x; 1.0076x over previous
"""Trainium2 Bass kernel for nn_CholeskyConstraintLayer.

Maps x:(B,16) f32 -> rho:(B,4,4,2) f32 where rho = L L^dagger / (trace + eps),
L lower-triangular complex 4x4 built from x (softplus diagonal, raw re/im
off-diagonals).

PLANAR (SoA) design: the host stages each (core, tile) block of samples as a
plane-major (P, 16, F) array -- a pure layout transpose -- so that on-chip
every operand is a stride-1 run of F samples.  That keeps every DVE
tensor_tensor in the 2x fp16 fast mode (the cost model requires the last AP
dim to be packed for ALL operands) and lets per-sample broadcasts (rcp, dr)
ride outer stride-0 AP dims, which do not break the fast mode.

x plane order (natural tri layout): [d0, r10,i10, d1, r20,i20, r21,i21, d2,
r30,i30, r31,i31, r32,i32, d3]; d* get softplus.  The stride-2 (r,i)
interleave gives regular AP patterns: R2=(r20,r21)@{4,6}, I2=(i20,i21)@{5,7},
R3=(r30,r31,r32)@{9,11,13}, I3=(i30,i31,i32)@{10,12,14}, R1=(r10,d1)@{1,3}.

Per-sample math (22 products, 31 adds, softplus, 16 squares, recip, 16 norm
muls, 6 negations):
  re21 = r20*r10 + r21*d1 + i20*i10       im21 = i20*r10 + i21*d1 - r20*i10
  re31/im31 analogous with row 3;         re32 = R3.(r20,r21,d2) + (i30,i31).I2
  im32 = I3.(r20,r21,d2) - (r30,r31).I2
  qii = row sums of squares; trace = q00+q11+q22+q33 (+eps); rho *= 1/trace
  col0: (re_i0, im_i0) = (r_i0, i_i0) * d0 / trace  via dr = d0*rcp

Engine split: DVE does products + add-trees + trace/reciprocal (fp32) +
rcp cast + dr/q00/col0; ACT does softplus (Exp,Ln), the 16 squares and the
col0-imag negations; Pool does the q/dots normalises (tensor_tensor with the
rcp broadcast riding an outer stride-0 dim), the dot-imag negations, and the
L2a/L2b dot finishers (their outputs feed only Pool, so deferring them to S3
takes them off the DVE critical path without an in-order-queue stall).
Emission is a 4-stage software pipeline, one tile per stage per round:
S0 dma-in | S1 ACT softplus+squares | S2 DVE compute | S3 Pool norm + negs +
dma-out (3 chunks for big tiles so draining starts early).  The last
TAIL_DVE tiles run their norms/negs on DVE so the kernel end is not
serialised behind Pool's queue.

Output is 22 fp16 planes per tile: [q00, re10,im10,re20,im20,re30,im30,
q11,q22,q33, re21,re31,re32, im21,im31,im32, nim10,nim20,nim30, nim21,
nim31,nim32].  The host only gathers/zero-fills these into the (B,4,4,2)
f32 layout; all arithmetic happens on device.
"""

import numpy as np

P = 128
EPS = 1e-8
N_CORES = 8
BATCH = 1_000_000
SPP = 977  # samples per partition; P*SPP*N_CORES = 1000448 >= BATCH
F_LIST = [180, 300, 275, 190, 32]  # sum = SPP; descending taper hides tails
TAIL_DVE = 2  # trailing tiles whose normalise/negs run on DVE instead of Pool
NEG_MODE = "post"   # 'post': nim = im_norm * -1 ; 'nrcp': nim = im_unnorm * -rcp
NEGC_ENGINE = "act"  # engine for col0-imag negation on non-tail tiles
CHUNK_A_MIN = 128    # min F for a separate early chunk-A out DMA
CHUNK_C = True       # separate chunk-C DMA for negated planes
COL0_POOL = 0        # how many col0 planes (0/4/6/7=incl q00) run on Pool
DMAQ_POOL = False    # issue mid-tile out-DMAs from Pool (SWDGE) vs SP (HWDGE)
SQ_BUFS = 3
OUT_BUFS = 3
SPLIT_IN0 = True     # split tile0's in-DMA into two plane chunks
SQ0_DVE = False      # tile0's squares on DVE instead of ACT
RCP32_DIRECT = False  # Pool norms read fp32 rcp directly (no fp16 cast)
M4_POOL = False      # M4a/M4b (32-dot finishers, Pool-only consumers) on Pool
L2_POOL = True       # L2a/L2b (21/31-dot finishers) on Pool
NEGD_DVE = False     # dot-imag negation on DVE (4x) instead of Pool
S_CORE = P * SPP
S_PAD = S_CORE * N_CORES

IN_W = 16   # fp16 planes per sample in
OUT_W = 22  # fp16 planes per sample out

# out22 plane order: [q00, re10,im10, re20,im20, re30,im30, q11,q22,q33,
#  re21,re31,re32, im21,im31,im32, nim10,nim20,nim30, nim21,nim31,nim32]
# out plane -> rho flat-32 expansion (host): rho32[k] = out22[EXP_SRC[k]],
# EXP_SRC=-1 -> 0.
EXP_SRC = np.full(32, -1, dtype=np.int64)
for flat, src in {
    0: 0, 10: 7, 20: 8, 30: 9,
    8: 1, 9: 2, 2: 1, 3: 16,
    16: 3, 17: 4, 4: 3, 5: 17,
    24: 5, 25: 6, 6: 5, 7: 18,
    18: 10, 19: 13, 12: 10, 13: 19,
    26: 11, 27: 14, 14: 11, 15: 20,
    28: 12, 29: 15, 22: 12, 23: 21,
}.items():
    EXP_SRC[flat] = src

_NC_CACHE = {}


def _emit(tc, x_ap, out_ap, f_list):
    import concourse.bass as bass
    import concourse.mybir as mybir
    from contextlib import ExitStack

    nc = tc.nc
    f16 = mybir.dt.float16
    f32 = mybir.dt.float32
    A = mybir.AluOpType
    ACT = mybir.ActivationFunctionType

    def pap(tile, F, p0, dims=()):
        """Plane-pattern AP on a (P, nplanes*F) tile: outer dims in plane
        units [stride, count], innermost packed [1, F]."""
        v = tile[:, :]
        return bass.AP(tensor=v.tensor, offset=v.offset + p0 * F,
                       ap=[list(v.ap[0])] + [[s * F, c] for s, c in dims]
                       + [[1, F]])

    with ExitStack() as ctx:
        tp = lambda name, bufs: ctx.enter_context(
            tc.tile_pool(name=name, bufs=bufs))
        wpool = tp("w", 4 if COL0_POOL else 3)  # S0..S2 (+Pool lag)
        sqpool = tp("sq", SQ_BUFS)  # alive S1..S2
        tppool = tp("tp", 3 if L2_POOL else 2)
        scpool = tp("sc", 3 if (M4_POOL or L2_POOL) else 2)
        qcpool = tp("qc", 3)   # S2..S3 (Q planes read by Pool N1)
        dqpool = tp("dq", 3)   # S2..S3: dots, rcp16, dr
        c32pool = tp("c32", 3 if RCP32_DIRECT else 2)
        opool = tp("out", OUT_BUFS)  # written S2 (col0) + S3

        offs = []
        o = 0
        for F in f_list:
            offs.append(o)
            o += F
        states = [dict(ti=i, F=f_list[i], off=offs[i]) for i in range(len(f_list))]

        def s0_dma_in(st):
            F, off = st["F"], st["off"]
            w_t = wpool.tile([P, 16 * F], f16, tag="w", name=f"w{st['ti']}")
            W = lambda p0, dims=(): pap(w_t, F, p0, dims)
            if st["ti"] == 0 and SPLIT_IN0:
                # split tile0's load so softplus(d0,d1) starts before the
                # full tile arrives (shorter pipeline fill)
                for p0, np_ in ((0, 8), (8, 8)):
                    xin = bass.AP(tensor=x_ap.tensor,
                                  offset=IN_W * off + p0 * F,
                                  ap=[[IN_W * SPP, P], [1, np_ * F]])
                    nc.sync.dma_start(W(p0, [[1, np_]]), xin)
            else:
                xin = bass.AP(tensor=x_ap.tensor, offset=IN_W * off,
                              ap=[[IN_W * SPP, P], [1, IN_W * F]])
                nc.sync.dma_start(w_t[:, :], xin)
            st["W"] = W

        def s1_act(st):
            F, W = st["F"], st["W"]
            # softplus in place on d-planes {0,3,8,15}
            # (exp scratch = sq planes 0..3, overwritten later by Square)
            sq_t = sqpool.tile([P, 16 * F], f16, tag="sq", name=f"sq{st['ti']}")
            SQ = lambda p0, dims=(): pap(sq_t, F, p0, dims)
            nc.scalar.activation(SQ(0, [[1, 2]]), W(0, [[3, 2]]), ACT.Exp)
            nc.scalar.activation(SQ(2, [[1, 2]]), W(8, [[7, 2]]), ACT.Exp)
            nc.scalar.activation(W(0, [[3, 2]]), SQ(0, [[1, 2]]), ACT.Ln,
                                 bias=1.0)
            nc.scalar.activation(W(8, [[7, 2]]), SQ(2, [[1, 2]]), ACT.Ln,
                                 bias=1.0)
            # squares of all 16 planes
            if st["ti"] == 0 and SQ0_DVE:
                nc.vector.tensor_tensor(SQ(0, [[1, 16]]), W(0, [[1, 16]]),
                                        W(0, [[1, 16]]), op=A.mult)
            else:
                nc.scalar.activation(SQ(0, [[1, 16]]), W(0, [[1, 16]]), ACT.Square)
            st["SQ"] = SQ

        def s2_dve(st):
            ti, F, W, SQ, off = st["ti"], st["F"], st["W"], st["SQ"], st["off"]
            tt = lambda dst, a, b, op: nc.vector.tensor_tensor(dst, a, b, op=op)
            # ---- products (22 els/sample), term planes TP[0:23]
            # TP: 0-7 Pa (re21 t0t1, im21 t0t1, re31 t0t1, im31 t0t1),
            #     8-13 Pb (re32 t0t1t2, im32 t0t1t2),
            #     14-18 Pd (im32s0@14, im32s1@15, [16 unused], re32e0@17, re32e1@18)
            #     19-22 Pc (im21s, re21t2, im31s, re31t2)
            tp_t = tppool.tile([P, 23 * F], f16, tag="tp", name=f"tp{ti}")
            TP = lambda p0, dims=(): pap(tp_t, F, p0, dims)
            # Pd, Pc first: no softplus dependency
            tt(TP(14, [[3, 2], [1, 2]]), W(9, [[1, 2], [2, 2]]), W(5, [[0, 2], [2, 2]]), A.mult)
            tt(TP(19, [[1, 4]]), W(4, [[5, 2], [1, 2]]), W(2, [[0, 2], [0, 2]]), A.mult)
            tt(TP(0, [[1, 4]]), W(4, [[1, 2], [2, 2]]), W(1, [[0, 2], [2, 2]]), A.mult)
            tt(TP(4, [[1, 4]]), W(9, [[1, 2], [2, 2]]), W(1, [[0, 2], [2, 2]]), A.mult)
            tt(TP(8, [[1, 6]]), W(9, [[1, 2], [2, 3]]), W(4, [[0, 2], [2, 3]]), A.mult)

            # ---- off-diag add tree -> dots dq[0:6]
            sc_np = 14 if L2_POOL else 10
            sc_t = scpool.tile([P, sc_np * F], f16, tag="sc", name=f"sc{ti}")
            SC = lambda p0, dims=(): pap(sc_t, F, p0, dims)
            dq_t = dqpool.tile([P, 9 * F], f16, tag="dq", name=f"dq{ti}")
            # dq planes: 0-5 dots (re21,re31,re32,im21,im31,im32), 6 rcp16,
            # 7 dr, 8 nrcp (= -rcp16, for the negated-imag outputs)
            DQ = lambda p0, dims=(): pap(dq_t, F, p0, dims)
            tail_t = ti >= len(f_list) - TAIL_DVE
            # L1: S[0:4] = (re21',im21',re31',im31')
            tt(SC(0, [[1, 4]]), TP(0, [[2, 4]]), TP(1, [[2, 4]]), A.add)
            if not (L2_POOL and not tail_t):
                tt(DQ(0, [[1, 2]]), SC(0, [[2, 2]]), TP(20, [[2, 2]]), A.add)
                tt(DQ(3, [[1, 2]]), SC(1, [[2, 2]]), TP(19, [[2, 2]]), A.subtract)
            # M13: (U0,U1,Vs,Ve) = TP{8,11,14,17} + TP{9,12,15,18}
            tt(SC(4, [[1, 4]]), TP(8, [[3, 4]]), TP(9, [[3, 4]]), A.add)
            # M2: U2 = U + (t2 of re32, im32)
            tt(SC(8, [[1, 2]]), SC(4, [[1, 2]]), TP(10, [[3, 2]]), A.add)
            if not (M4_POOL and not tail_t):
                tt(DQ(2), SC(8), SC(7), A.add)        # re32 = re32a + Ve
                tt(DQ(5), SC(9), SC(6), A.subtract)   # im32 = im32a - Vs
            st["SC"], st["TPf"] = SC, TP

            # ---- diag add tree (reuses sc planes 0..3 for B)
            # qc: 0 q11', 1 q22', 2 q33', 3 E, 4 q33p, 5 t1, 6 t2,
            #     7 q11, 8 q22, 9 q33
            qc_t = qcpool.tile([P, 10 * F], f16, tag="qc", name=f"qc{ti}")
            QC = lambda p0, dims=(): pap(qc_t, F, p0, dims)
            b0 = 10 if L2_POOL else 0  # avoid clobbering S before Pool's L2
            tt(SC(b0, [[1, 4]]), SQ(4, [[5, 2], [1, 2]]), SQ(6, [[5, 2], [1, 2]]), A.add)
            tt(QC(0, [[3, 2]]), SQ(1, [[12, 2]]), SQ(2, [[12, 2]]), A.add)
            tt(QC(1, [[1, 2]]), SC(b0, [[2, 2]]), SC(b0 + 1, [[2, 2]]), A.add)
            tt(QC(7, [[1, 2]]), QC(0, [[1, 2]]), SQ(3, [[5, 2]]), A.add)
            # fused: (q33p, t1) = (q33', q11) + (E, q22)
            tt(QC(4, [[1, 2]]), QC(2, [[5, 2]]), QC(3, [[5, 2]]), A.add)
            tt(QC(9), QC(4), SQ(15), A.add)
            tt(QC(6), QC(9), SQ(0), A.add)

            # ---- trace -> rcp (fp32), cast fp16 on ACT, dr + col0 on DVE
            c32_t = c32pool.tile([P, 2 * F], f32, tag="c32", name=f"c{ti}")
            trE = pap(c32_t, F, 0)
            rcp32 = pap(c32_t, F, 1)
            nc.vector.scalar_tensor_tensor(trE, QC(5), float(EPS), QC(6),
                                           op0=A.add, op1=A.add)
            nc.vector.reciprocal_approx_fast(rcp32, trE)
            if RCP32_DIRECT:
                # Pool has no fast mode, so its norms read rcp32 directly;
                # dr drops to 1x (mixed dtype) but the cast disappears.
                RCP = lambda dims=(): pap(c32_t, F, 1, dims)
                tt(DQ(7), W(0), RCP(), A.mult)  # dr = d0 * rcp (1x)
            else:
                RCP = lambda dims=(): pap(dq_t, F, 6, dims)
                nc.vector.tensor_copy(DQ(6), rcp32)
                tt(DQ(7), W(0), DQ(6), A.mult)  # dr = d0 * rcp
            if NEG_MODE == "nrcp":
                nc.vector.tensor_scalar_mul(DQ(8), RCP(), -1.0)  # nrcp
            st["RCP"] = RCP
            out_t = opool.tile([P, OUT_W * F], f16, tag="out", name=f"o{ti}")
            OUT = lambda p0, dims=(): pap(out_t, F, p0, dims)
            tail = ti >= len(f_list) - TAIL_DVE
            c0p = 0 if tail else COL0_POOL
            # (q00, re10, im10) = (d0, r10, i10) * dr ; then rows 2,3 pairs
            # (planes moved to Pool run in s3 to avoid queue head-of-line)
            if c0p < 7:
                tt(OUT(0, [[1, 3]]), W(0, [[1, 3]]), DQ(7, [[0, 3]]), A.mult)
            if c0p < 4:
                tt(OUT(3, [[1, 4]]), W(4, [[5, 2], [1, 2]]),
                   DQ(7, [[0, 2], [0, 2]]), A.mult)
            st["c0p"] = c0p
            if F >= CHUNK_A_MIN and not c0p:
                # DMA out chunk A: DVE-written planes 0..6
                odst = bass.AP(tensor=out_ap.tensor, offset=OUT_W * off,
                               ap=[[OUT_W * SPP, P], [1, 7 * F]])
                nc.sync.dma_start(odst, OUT(0, [[1, 7]]))
                st["chunkA"] = True
            else:
                st["chunkA"] = False
            st["DQ"], st["QC"], st["OUT"] = DQ, QC, OUT
            st["Wf"] = W

        def s3_tail(st):
            F, off = st["F"], st["off"]
            DQ, QC, OUT = st["DQ"], st["QC"], st["OUT"]
            tail = st["ti"] >= len(f_list) - TAIL_DVE
            if tail:
                # last tiles: run norms on DVE so the kernel end is not
                # serialised behind Pool's queue
                ntt = lambda dst, a, b: nc.vector.tensor_tensor(dst, a, b, op=A.mult)
                nts = nc.vector.tensor_scalar_mul
            else:
                ntt = lambda dst, a, b: nc.gpsimd.tensor_tensor(dst, a, b, op=A.mult)
                nts = nc.gpsimd.tensor_scalar_mul
            # col0 planes assigned to Pool (emitted here, not in s2)
            c0p, W, F_ = st["c0p"], st["Wf"], st["F"]
            gtt = lambda dst, a, b: nc.gpsimd.tensor_tensor(dst, a, b, op=A.mult)
            SC, TP = st["SC"], st["TPf"]
            gta = lambda dst, a, b, op: nc.gpsimd.tensor_tensor(dst, a, b, op=op)
            if L2_POOL and not tail:
                gta(DQ(0, [[1, 2]]), SC(0, [[2, 2]]), TP(20, [[2, 2]]), A.add)
                gta(DQ(3, [[1, 2]]), SC(1, [[2, 2]]), TP(19, [[2, 2]]), A.subtract)
            if M4_POOL and not tail:
                gta(DQ(2), SC(8), SC(7), A.add)
                gta(DQ(5), SC(9), SC(6), A.subtract)
            if c0p >= 7:
                gtt(OUT(0, [[1, 3]]), W(0, [[1, 3]]), DQ(7, [[0, 3]]))
            if c0p >= 4:
                gtt(OUT(3, [[1, 4]]), W(4, [[5, 2], [1, 2]]),
                    DQ(7, [[0, 2], [0, 2]]))
            # q-norm + dots-norm (rcp bcast in outer stride-0 dim)
            RCP = st["RCP"]
            ntt(OUT(7, [[1, 3]]), QC(7, [[1, 3]]), RCP([[0, 3]]))
            ntt(OUT(10, [[1, 6]]), DQ(0, [[1, 6]]), RCP([[0, 6]]))
            # chunk B: normalised planes 7..15 (0..15 if no chunk A)
            p0 = 7 if st["chunkA"] else 0
            chunk_c = (CHUNK_C == 2) or (CHUNK_C and st["ti"] < len(f_list) - 1)
            pend = OUT_W if not chunk_c else 16
            dmaq = nc.scalar if tail else (nc.gpsimd if DMAQ_POOL else nc.sync)
            odstB = bass.AP(tensor=out_ap.tensor, offset=OUT_W * off + p0 * F,
                            ap=[[OUT_W * SPP, P], [1, (pend - p0) * F]])
            dmaB = lambda: dmaq.dma_start(odstB, OUT(p0, [[1, pend - p0]]))
            if chunk_c:
                dmaB()
            # negations
            if NEG_MODE == "nrcp":
                ntt(OUT(19, [[1, 3]]), DQ(3, [[1, 3]]), DQ(8, [[0, 3]]))
            elif NEGD_DVE:
                nc.vector.tensor_scalar_mul(OUT(19, [[1, 3]]), OUT(13, [[1, 3]]), -1.0)
            else:
                nts(OUT(19, [[1, 3]]), OUT(13, [[1, 3]]), -1.0)
            if tail:
                nc.vector.tensor_scalar_mul(OUT(16, [[1, 3]]), OUT(2, [[2, 3]]), -1.0)
            elif NEGC_ENGINE == "act":
                nc.scalar.mul(OUT(16, [[1, 3]]), OUT(2, [[2, 3]]), -1.0)
            else:
                nc.gpsimd.tensor_scalar_mul(OUT(16, [[1, 3]]), OUT(2, [[2, 3]]), -1.0)
            if chunk_c:
                odstC = bass.AP(tensor=out_ap.tensor, offset=OUT_W * off + 16 * F,
                                ap=[[OUT_W * SPP, P], [1, 6 * F]])
                dmaq.dma_start(odstC, OUT(16, [[1, 6]]))
            else:
                dmaB()

        nt = len(f_list)
        for r in range(nt + 3):
            if r < nt:
                s0_dma_in(states[r])
            if 1 <= r < nt + 1:
                s1_act(states[r - 1])
            if 2 <= r < nt + 2:
                s2_dve(states[r - 2])
            if 3 <= r:
                s3_tail(states[r - 3])


def _patch_act_tables():
    """Force every ACT function onto one table set so the table-load pass
    emits a single load (Exp/Ln/Square/Copy are all natively co-resident in
    natural_log_exp_and_others -- verified by the harness rel-err check)."""
    import concourse.bacc as bacc
    from concourse.hw_specs import get_activation_tables as _orig

    if getattr(bacc, "_act_tables_patched", False):
        return

    def _patched(arch):
        t = _orig(arch)
        return {k: (v if k == "natural_log_exp_and_others" else set())
                for k, v in t.items()}

    bacc.get_activation_tables = _patched
    bacc._act_tables_patched = True


def _build_nc(f_list):
    import concourse.bacc as bacc
    import concourse.mybir as mybir
    import concourse.tile as tile

    _patch_act_tables()

    key = (tuple(f_list), TAIL_DVE, NEG_MODE, NEGC_ENGINE, CHUNK_A_MIN, CHUNK_C,
           COL0_POOL, DMAQ_POOL, SQ_BUFS, OUT_BUFS, SPLIT_IN0, SQ0_DVE,
           RCP32_DIRECT, M4_POOL, L2_POOL, NEGD_DVE)
    if key in _NC_CACHE:
        return _NC_CACHE[key]
    nc = bacc.Bacc("TRN2", target_bir_lowering=False, debug=False)
    x = nc.dram_tensor("x", (P, IN_W * SPP), mybir.dt.float16,
                       kind="ExternalInput")
    out = nc.dram_tensor("out", (P, OUT_W * SPP), mybir.dt.float16,
                         kind="ExternalOutput")
    with tile.TileContext(nc) as tc:
        with nc.allow_low_precision(reason="fp16 pipeline, rel-err budget 2e-2"):
            _emit(tc, x.ap(), out.ap(), f_list)
    nc.compile()
    _NC_CACHE[key] = nc
    return nc


def _stage_in(x):
    """(B,16) f32 -> per-core (P, 16*SPP) fp16, per-tile plane-major blocks.
    Pure layout (pad, reshape, transpose) + fp16 cast."""
    B = x.shape[0]
    xp = np.zeros((S_PAD, IN_W), dtype=np.float16)
    xp[:B] = x
    xr = xp.reshape(N_CORES, P, SPP, IN_W)
    parts = []
    off = 0
    for F in F_LIST:
        blk = xr[:, :, off:off + F, :].transpose(0, 1, 3, 2)
        parts.append(np.ascontiguousarray(blk).reshape(N_CORES, P, IN_W * F))
        off += F
    return np.concatenate(parts, axis=2)


def _unstage_out(res_list, B):
    """Per-core (P, 22*SPP) fp16 tile blocks -> (B, 4, 4, 2) f32 via the
    EXP_SRC gather (host does layout + zero-fill only)."""
    out = np.stack([r.reshape(P, OUT_W * SPP) for r in res_list], axis=0)
    parts = []
    off = 0
    for F in F_LIST:
        blk = out[:, :, OUT_W * off:OUT_W * (off + F)]
        blk = blk.reshape(N_CORES, P, OUT_W, F).transpose(0, 1, 3, 2)
        parts.append(blk)
        off += F
    o22 = np.concatenate(parts, axis=2).reshape(S_PAD, OUT_W)[:B]
    out32 = np.zeros((B, 32), dtype=np.float32)
    used = EXP_SRC >= 0
    out32[:, used] = o22[:, EXP_SRC[used]].astype(np.float32)
    return out32.reshape(B, 4, 4, 2)


def kernel(x, _trace=False):
    from concourse.bass_utils import run_bass_kernel_spmd

    x = np.ascontiguousarray(np.asarray(x, dtype=np.float32))
    B = x.shape[0]
    assert x.shape == (B, 16) and B <= S_PAD
    xs = _stage_in(x)
    nc = _build_nc(F_LIST)
    in_maps = [{"x": np.ascontiguousarray(xs[i])} for i in range(N_CORES)]
    res = run_bass_kernel_spmd(nc, in_maps, core_ids=list(range(N_CORES)),
                               trace=_trace)
    result = _unstage_out([r["out"] for r in res.results], B)
    if _trace:
        return result, res
    return result
